# revision 8
# baseline (speedup 1.0000x reference)
"""Trainium2 Bass kernel for nn_Attention3D: RMSNorm3D + 8-head attention + out-proj.

Sharding: 16 (b, h) slices over 8 cores -> each core gets one batch b and two
heads. The host folds g*sqrt(C) and the dh^-0.5 scale into the projection
weights, normalizes x columns (the RMSNorm division), slices per-core weights,
and converts x/w_qkv/w_v to bf16; the device does the projections, attention,
and the partial output projection; the host sums the 4 partial y's per batch.

Device design notes (the ScalarE exp stream, 33.5M exps/core, is the
roofline; everything else hides under it):
 - One continuous exp stream over 512 (stage, key-tile) iterations grouped
   3-per-unit into [128, 3, 512] PSUM slabs: 1536-wide exp instructions
   amortize ScalarE's per-instruction SBUF-access overhead, double-buffered
   in a dedicated 6-bank pool so scores never contend with setup traffic.
 - Attention@V keeps p (bf16) stationary via LDWEIGHTS and streams v 64-wide
   plus a ones column, accumulating out[q, dh] and the softmax denominator
   per query partition in one padded 1-bank accumulator; the denominator
   divide is a per-partition broadcast multiply on VectorE.
 - All setup (q/k/v projections) and output (transpose via PE, out-proj,
   DMA) work flows through a single 1-bank PSUM ring, scheduled
   deadline-first so no pass ever blocks the exp stream.
 - Stage outputs transpose through the PE (o^T needed by the out-proj);
   the final stage's chain is phase-per-engine so the drain tail stays short.
"""
from contextlib import ExitStack

import numpy as np

import concourse.bass as bass
import concourse.tile as tile
from concourse import bacc, mybir
from concourse.bass_utils import run_bass_kernel_spmd

F32 = mybir.dt.float32
F32R = mybir.dt.float32r
BF16 = mybir.dt.bfloat16
AF = mybir.ActivationFunctionType

B, C, H, W, D = 2, 256, 16, 16, 16
N = H * W * D            # 4096
HEADS, DH = 8, 64
HID = HEADS * DH
NCORES = 8

NJ = N // 128            # 32 key tiles per stage
NSTAGES = 16             # (ic, h): 8 i-chunks x 2 heads
ICH = 512                # queries per stage
NQC = ICH // 128         # 4 query chunks
TOTJ = NSTAGES * NJ      # 512
JPU = 3                  # key tiles per exp unit ([128, 3, 512] slabs)
AV_LAG = 2               # units between exp and its attention@V consumption


def _stage_of(jg):
    s = jg // NJ
    return s // 2, s % 2, jg % NJ  # ic, h, jloc


def build_body(nc, tc, ctx, xb, wa, wv, wo, eye, y, yb):
    const = ctx.enter_context(tc.tile_pool(name="const", bufs=1))
    work = ctx.enter_context(tc.tile_pool(name="work", bufs=2))
    # PSUM pools in creation order; all sizes are whole banks:
    #  psA: scores [128,3,512] f32 x2 bufs (double buffered) -> 6 banks
    #  psS: setup/output ring [128,512] -> 1 bank (never touched by scores)
    #  psO: o accumulators (64 dims + denom col, padded) [128,4,128] -> 1 bank
    psA = ctx.enter_context(tc.tile_pool(name="psA", bufs=1, space="PSUM"))
    psS = ctx.enter_context(tc.tile_pool(name="psS", bufs=1, space="PSUM"))
    psO = ctx.enter_context(tc.tile_pool(name="psO", bufs=1, space="PSUM"))

    x_sb = const.tile([128, 2, N], BF16, tag="x")           # [c-tile, ct, n]
    wa_sb = const.tile([128, 2, 256], BF16, tag="wa")       # [c, (q128|k128)]
    wv_sb = const.tile([128, 2, 128], BF16, tag="wv")       # [c, 128]
    wo_sb = const.tile([64, 2, 256], F32R, tag="wo")        # [dh, h, c]
    eye_sb = const.tile([128, 128], F32R, tag="eye")
    ones_col = const.tile([128, 1], F32R, tag="onesc")
    ones_row = const.tile([1, 128], F32R, tag="onesr")
    qk_slab = const.tile([128, 2, N], F32R, tag="qk")       # [(h,dh), q/k, n]
    v_slab = const.tile([128, NJ, 2, 65], BF16, tag="v")    # [key, t, h, dh+1]

    ONE = 0x3F800000
    U32 = mybir.dt.uint32
    nc.vector.memset(ones_col[:].bitcast(U32), ONE)
    nc.vector.memset(ones_row[:].bitcast(U32), ONE)
    U16 = mybir.dt.uint16
    nc.vector.memset(v_slab[:, :, :, 64:65].bitcast(U16), 0x3F80)

    # Load the exp table up front so neither the load nor any implicit
    # insertion lands on the first-exp critical path (Copy lives in the
    # same set, so this is the only load in the program).
    from concourse.hw_specs import get_activation_tables
    tabs = get_activation_tables(nc.m.arch)
    exp_id = next(i for i, (tname, funcs) in enumerate(tabs.items())
                  if AF.Exp in funcs and AF.Copy in funcs)
    nc.scalar.add_instruction(mybir.InstLoadActFuncSet(
        name=nc.get_next_instruction_name(), act_func_set_id=exp_id,
        ins=[], outs=[]))


    # ---- input DMAs: weights first (qk projections gate the first exp),
    # then x in three bundles per c-tile split across both queues ----
    for ct in range(2):
        nc.gpsimd.dma_start(out=wa_sb[:, ct, :], in_=wa[ct * 128:(ct + 1) * 128, :])
    for lo, hi in ((0, 1024), (1024, 1536)):
        nc.sync.dma_start(out=x_sb[:, 0, lo:hi], in_=xb[0:128, lo:hi])
        nc.sync.dma_start(out=x_sb[:, 1, lo:hi], in_=xb[128:256, lo:hi])
    for ct in range(2):
        nc.gpsimd.dma_start(out=wv_sb[:, ct, :], in_=wv[ct * 128:(ct + 1) * 128, :])
    nc.gpsimd.dma_start(out=eye_sb[:], in_=eye)
    for lo, hi in ((1536, 2560), (2560, 4096)):
        nc.sync.dma_start(out=x_sb[:, 0, lo:hi], in_=xb[0:128, lo:hi])
        nc.sync.dma_start(out=x_sb[:, 1, lo:hi], in_=xb[128:256, lo:hi])

    def wo_dma():
        nc.sync.dma_start(out=wo_sb[:, :, :],
                          in_=wo.rearrange("(d h) c -> d h c", h=2))

    # ---- projection bundles; chunk ch = columns [512ch, 512ch+512) ----
    def qk_bundle(ch, which):
        """q or k projection for one full 512-col chunk."""
        def emit():
            sl = bass.ts(ch, 512)
            col = slice(0, 128) if which == "q" else slice(128, 256)
            ps = psS.tile([128, 512], F32, tag="ss", name=f"{which}p_{ch}")
            for ct in range(2):
                nc.tensor.matmul(ps[:], wa_sb[:, ct, col],
                                 x_sb[:, ct, sl], start=(ct == 0), stop=(ct == 1))
            slot = 0 if which == "q" else 1
            nc.vector.tensor_copy(qk_slab[:, slot, sl], ps[:])
        return emit

    def qk_pair_prefix():
        """q ch0 + k ch0 + k ch1 through a single slab pass (prefix fast
        path): ct0 matmuls first so they start when the first x half lands;
        copies split DVE/ScalarE."""
        ps = psA.tile([128, 3, 512], F32, tag="s", bufs=2, name="qkpre")
        trio = [("q", 0, ps[:, 0, :]), ("k", 0, ps[:, 1, :]),
                ("k", 1, ps[:, 2, :])]
        for ct in range(2):
            for which, ch, view in trio:
                col = slice(0, 128) if which == "q" else slice(128, 256)
                sl = bass.ts(ch, 512)
                nc.tensor.matmul(view, wa_sb[:, ct, col], x_sb[:, ct, sl],
                                 start=(ct == 0), stop=(ct == 1),
                                 skip_group_check=True)
        for i, (which, ch, view) in enumerate(trio):
            sl = bass.ts(ch, 512)
            slot = 0 if which == "q" else 1
            if i == 1:
                nc.scalar.copy(qk_slab[:, slot, sl], view)
            else:
                nc.vector.tensor_copy(qk_slab[:, slot, sl], view)

    def v_bundle(ts):
        """stage V (both heads, bf16) for 4 key tiles (bf16 matmuls run at
        full rate below 256 cols, so the projection is 128 wide)."""
        def emit():
            t0 = ts[0]
            ps = psS.tile([128, 4, 128], F32, tag="ss", name=f"vp_{t0}")
            for i, t in enumerate(ts):
                for ct in range(2):
                    nc.tensor.matmul(ps[:, i, :],
                                     x_sb[:, ct, t * 128:(t + 1) * 128],
                                     wv_sb[:, ct, :],
                                     start=(ct == 0), stop=(ct == 1))
            nc.vector.tensor_copy(v_slab[:, t0:t0 + 4, :, 0:64], ps[:])
        return emit

    # ---- global unit list: 3 key half-tiles per unit, 1536-col exps ----
    units = []
    jg = 0
    while jg < TOTJ:
        take = min(JPU, TOTJ - jg)
        units.append(list(range(jg, jg + take)))
        jg += take
    NU = len(units)

    deferred = {}

    def defer(u, fn):
        deferred.setdefault(max(0, min(u, NU - 1)), []).append(fn)

    # ---- prefix ----
    warm = psS.tile([128, 512], F32, tag="ss", name="warm")
    for i in range(8):
        nc.tensor.matmul(warm[:, 128 * (i % 4):128 * (i % 4) + 128],
                         ones_row[:], ones_row[:])
    qk_pair_prefix()
    v_bundle([0, 1, 2, 3])()
    v_bundle([4, 5, 6, 7])()
    qk_bundle(2, "k")()
    v_bundle([8, 9, 10, 11])()

    # deferred chunk pipelines: EDF over the single misc bank. A k half
    # (ch, half) gates scores(jl=4ch+2half); a v pair gates its AV two
    # units later. Greedy earliest-slot assignment, allowing doubles only
    # in the first two units (the stream is still filling then).
    unit_of = {}
    for ui, js_ in enumerate(units):
        for j_ in js_:
            unit_of[j_] = ui
    items = []
    for ch in range(3, 8):
        items.append((unit_of[4 * ch] - 2, "k", ch))
    for ch in range(3, 8):
        items.append((unit_of[4 * ch] + AV_LAG - 1, "v", ch))
    items.sort(key=lambda it: it[0])
    used = {}
    for dl, kind, ch in items:
        u = 0
        while u < dl and used.get(u, 0) >= 1:
            u += 1
        used[u] = used.get(u, 0) + 1
        if kind == "k":
            defer(u, qk_bundle(ch, "k"))
        else:
            defer(u, v_bundle([4 * ch, 4 * ch + 1, 4 * ch + 2, 4 * ch + 3]))
    defer(0, wo_dma)
    for ch in range(1, 8):
        u0 = max(3, (64 * ch) // 3 - 14)
        defer(u0, qk_bundle(ch, "q"))

    # ---- per-stage state ----
    stage_state = {}
    ic_state = {}

    def get_stage(s):
        if s not in stage_state:
            ic, h = s // 2, s % 2
            if ic not in ic_state:
                osl = work.tile([128, NQC, 2, 64], F32R, tag="osl",
                                name=f"osl_{ic}")
                ot = work.tile([64, 2, ICH], F32R, tag="ot", name=f"ot_{ic}")
                ic_state[ic] = (osl, ot)
            o_ps = psO.tile([128, NQC, 128], F32, tag="o", name=f"o_{s}")
            stage_state[s] = (o_ps,) + ic_state[ic]
        return stage_state[s]

    def finalize(s):
        ic, h = s // 2, s % 2
        o_ps, osl, ot = stage_state[s]
        recd = work.tile([128, NQC, 1], F32, tag="rd", name=f"rd_{s}")
        nc.vector.reciprocal(out=recd[:], in_=o_ps[:, :, 64:65])
        nc.vector.tensor_mul(osl[:, :, h, :], o_ps[:, :, 0:64],
                             recd[:].broadcast_to([128, NQC, 64]))

    def transpose_bundle(ic, h, qcs):
        def emit():
            osl, ot = ic_state[ic]
            ps = psS.tile([128, 512], F32R, tag="ss",
                          name=f"tro_{ic}_{h}_{qcs[0]}")
            for i, qc in enumerate(qcs):
                view = ps[0:64, i * 128:(i + 1) * 128]
                nc.tensor.transpose(view, osl[:, qc, h, :], eye_sb[:])
            for i, qc in enumerate(qcs):
                view = ps[0:64, i * 128:(i + 1) * 128]
                nc.vector.tensor_copy(ot[:, h, qc * 128:(qc + 1) * 128], view)
        return emit

    def outproj_one(ic, cc):
        def emit():
            osl, ot = ic_state[ic]
            ps = psS.tile([128, 512], F32, tag="ss", name=f"yp_{ic}_{cc}")
            for mt in range(2):
                view = ps[:, mt * 256:mt * 256 + 256]
                for h in range(2):
                    nc.tensor.matmul(
                        view, wo_sb[:, h, mt * 128:(mt + 1) * 128],
                        ot[:, h, cc * 256:(cc + 1) * 256],
                        start=(h == 0), stop=(h == 1))
            for mt in range(2):
                view = ps[:, mt * 256:mt * 256 + 256]
                y_ev = work.tile([128, 256], F32, tag="yev", bufs=4,
                                 name=f"ye_{ic}_{cc}_{mt}")
                nc.vector.tensor_copy(y_ev[:], view)
                (nc.sync if mt == 0 else nc.gpsimd).dma_start(
                    out=y[mt * 128:(mt + 1) * 128,
                          ic * ICH + cc * 256:ic * ICH + (cc + 1) * 256],
                    in_=y_ev[:])
        return emit

    def outproj_bundle(ic, ccs):
        def emit():
            osl, ot = ic_state[ic]
            ps = psS.tile([128, 1024], F32, tag="ss", name=f"yp_{ic}_{ccs[0]}")
            for i, cc in enumerate(ccs):
                for mt in range(2):
                    view = ps[:, (2 * i + mt) * 256:(2 * i + mt) * 256 + 256]
                    for h in range(2):
                        nc.tensor.matmul(
                            view, wo_sb[:, h, mt * 128:(mt + 1) * 128],
                            ot[:, h, cc * 256:(cc + 1) * 256],
                            start=(h == 0), stop=(h == 1))
            for i, cc in enumerate(ccs):
                for mt in range(2):
                    view = ps[:, (2 * i + mt) * 256:(2 * i + mt) * 256 + 256]
                    y_ev = work.tile([128, 256], F32, tag="yev", bufs=4,
                                     name=f"ye_{ic}_{cc}_{mt}")
                    nc.vector.tensor_copy(y_ev[:], view)
                    nc.sync.dma_start(
                        out=y[mt * 128:(mt + 1) * 128,
                              ic * ICH + cc * 256:ic * ICH + (cc + 1) * 256],
                        in_=y_ev[:])
        return emit

    def emit_scores(u):
        js = units[u]
        s_ps = psA.tile([128, 3, 512], F32, tag="s", bufs=2, name=f"s_{u}")
        for i, j in enumerate(js):
            ic, h, jl = _stage_of(j)
            hsl = slice(h * 64, (h + 1) * 64)
            nc.tensor.matmul(
                s_ps[:, i, :],
                qk_slab[hsl, 1, jl * 128:(jl + 1) * 128],
                qk_slab[hsl, 0, ic * ICH:(ic + 1) * ICH])
        return s_ps

    cur_u = [0]

    def emit_av(js, p_views):
        for i, j in enumerate(js):
            s = j // NJ
            ic, h, jl = _stage_of(j)
            o_ps, osl, ot = get_stage(s)
            p = p_views[i]
            for qc in range(NQC):
                pw = p[:, qc * 128:(qc + 1) * 128]
                # one accumulation group for the whole o bank: the first
                # start pending-zeroes it; later qc first-writes land via
                # the per-element has_written bits.
                nc.tensor.matmul(o_ps[:, qc, 0:65], pw,
                                 v_slab[:, jl, h, :],
                                 start=(jl == 0 and qc == 0),
                                 stop=(jl == NJ - 1 and qc == NQC - 1),
                                 skip_group_check=True)
            if jl == NJ - 1 and s < NSTAGES - 1:
                finalize(s)
                u = cur_u[0]
                for i in range(2):
                    defer(u + 2 + 2 * i, transpose_bundle(ic, h,
                                                          [2 * i, 2 * i + 1]))
                if h == 1:
                    for cc in range(2):
                        defer(u + 6 + 3 * cc, outproj_one(ic, cc))

    # ---- main stream ----
    pending = []
    s_cur = emit_scores(0)
    for u in range(NU):
        cur_u[0] = u
        js = units[u]
        p_t = work.tile([128, 3, 512], BF16, tag="p", bufs=4, name=f"p_{u}")
        if len(js) == 3:
            nc.scalar.activation(out=p_t[:], in_=s_cur[:], func=AF.Exp)
        else:
            nc.scalar.activation(out=p_t[:, 0:len(js), :],
                                 in_=s_cur[:, 0:len(js), :], func=AF.Exp)
        p_views = [p_t[:, i, :] for i in range(len(js))]
        if u + 1 < NU:
            s_cur = emit_scores(u + 1)
        pending.append((js, p_views))
        if len(pending) > AV_LAG:
            emit_av(*pending.pop(0))
        for fn in deferred.pop(u, []):
            fn()
    while pending:
        emit_av(*pending.pop(0))
    for ukey in sorted(deferred):
        for fn in deferred.pop(ukey):
            fn()

    # engine-major tail for the last stage (s=15, ic=7, h=1).
    s = NSTAGES - 1
    ic = s // 2
    o_ps, osl, ot = stage_state[s]
    recd = work.tile([128, NQC, 1], F32, tag="rd", name="rd_tail")
    nc.vector.reciprocal(out=recd[:], in_=o_ps[:, :, 64:65])
    to = work.tile([128, NQC, 64], F32R, tag="to", bufs=2, name="to_t")
    nc.vector.tensor_mul(to[:], o_ps[:, :, 0:64],
                         recd[:].broadcast_to([128, NQC, 64]))
    trp = psS.tile([128, 512], F32R, tag="ss", name="ttr")
    trv = [trp[0:64, qc * 128:(qc + 1) * 128] for qc in range(NQC)]
    for qc in range(NQC):
        nc.tensor.transpose(trv[qc], to[:, qc, :], eye_sb[:])
    tot = [work.tile([64, 256], F32R, tag="tot", bufs=2, name=f"tot_{cc}")
           for cc in range(2)]
    for cc in range(2):
        if cc == 0:
            nc.vector.tensor_copy(tot[cc][:], trp[0:64, 0:256])
        else:
            nc.scalar.copy(tot[cc][:], trp[0:64, 256:512])
    yp1 = psA.tile([128, 3, 512], F32, tag="s", bufs=2, name="ty0")
    yviews = {}
    for mt in range(2):
        view = yp1[:, mt, :]
        yviews[mt] = view
        nc.tensor.matmul(view, wo_sb[:, 0, mt * 128:(mt + 1) * 128],
                         ot[:, 0, :], start=True, stop=False,
                         skip_group_check=True)
        for i in range(2):
            nc.tensor.matmul(view[:, i * 256:(i + 1) * 256],
                             wo_sb[:, 1, mt * 128:(mt + 1) * 128],
                             tot[i][:], start=False, stop=(i == 1),
                             skip_group_check=True)
    for mt in range(2):
        y_ev = work.tile([128, 512], BF16, tag="tyev", bufs=4,
                         name=f"tye_{mt}")
        if mt == 0:
            nc.vector.tensor_copy(y_ev[:], yviews[mt])
        else:
            nc.scalar.copy(y_ev[:], yviews[mt])
        (nc.sync if mt == 0 else nc.gpsimd).dma_start(
            out=yb[mt * 128:(mt + 1) * 128, :], in_=y_ev[:])


_NC_CACHE = None


def _build():
    global _NC_CACHE
    if _NC_CACHE is not None:
        return _NC_CACHE
    nc = bacc.Bacc("TRN2", target_bir_lowering=False, debug=False,
                   num_devices=NCORES)
    xb = nc.dram_tensor("xb", [C, N], BF16, kind="ExternalInput").ap()
    wa = nc.dram_tensor("wa", [C, 256], BF16, kind="ExternalInput").ap()
    wv = nc.dram_tensor("wv", [C, 128], BF16, kind="ExternalInput").ap()
    wo = nc.dram_tensor("wo", [128, C], F32R, kind="ExternalInput").ap()
    eye = nc.dram_tensor("eye", [128, 128], F32R, kind="ExternalInput").ap()
    y = nc.dram_tensor("y", [C, N], F32, kind="ExternalOutput").ap()
    yb = nc.dram_tensor("yb", [C, ICH], BF16, kind="ExternalOutput").ap()
    with tile.TileContext(nc) as tc, ExitStack() as ctx:
        with nc.allow_low_precision(reason="float32r rounding is within tolerance"):
            build_body(nc, tc, ctx, xb, wa, wv, wo, eye, y, yb)
    nc.compile()
    _NC_CACHE = nc
    return nc


def _host_prep(x, g, w_qkv, w_out):
    x = np.ascontiguousarray(np.asarray(x, np.float32))
    g = np.asarray(g, np.float32)
    w_qkv = np.asarray(w_qkv, np.float32)
    w_out = np.asarray(w_out, np.float32)

    Wg = w_qkv * (g * np.sqrt(np.float32(C)))[None, :]
    Wq = Wg[0:HID] * np.float32(DH ** -0.5)
    Wk = Wg[HID:2 * HID]
    Wv = Wg[2 * HID:3 * HID]
    eye = np.eye(128, dtype=np.float32)

    in_maps = []
    for core in range(NCORES):
        b = core // 4
        h0 = 2 * (core % 4)
        sl = slice(h0 * DH, (h0 + 2) * DH)
        W_A = np.concatenate([Wq[sl], Wk[sl]], 0)            # [256, 256]
        wo_slice = w_out[:, sl]                              # [256, 128]
        wo_dev = np.ascontiguousarray(
            wo_slice.T.reshape(2, DH, C).transpose(1, 0, 2).reshape(128, C))
        import ml_dtypes
        bf16 = ml_dtypes.bfloat16
        xc = x[b].reshape(C, N)
        xn = xc / np.maximum(np.sqrt((xc * xc).sum(0, keepdims=True)),
                             np.float32(1e-12))
        in_maps.append({
            "xb": np.ascontiguousarray(xn.astype(bf16)),
            "wa": np.ascontiguousarray(W_A.T.astype(bf16)),  # [c, o]
            "wv": np.ascontiguousarray(Wv[sl].T.astype(bf16)),
            "wo": wo_dev,                                    # [(d,h), c]
            "eye": eye,
        })
    return in_maps


_RUNNER_CACHE = None


def _make_runner(nc):
    import jax
    from jax.sharding import Mesh, PartitionSpec
    from jax.experimental.shard_map import shard_map
    from concourse import bass2jax

    bass2jax.install_neuronx_cc_hook()
    in_names, out_names, out_avals, zero_outs = [], [], [], []
    for alloc in nc.m.functions[0].allocations:
        if not isinstance(alloc, mybir.MemoryLocationSet):
            name = alloc.memorylocations[0].name
        if alloc.kind == "ExternalInput":
            if nc.partition_id_tensor is None or name != nc.partition_id_tensor.name:
                in_names.append(name)
        elif alloc.kind == "ExternalOutput":
            out_names.append(name)
            shape = tuple(alloc.tensor_shape)
            dtype = mybir.dt.np(alloc.dtype)
            out_avals.append(jax.core.ShapedArray(shape, dtype))
            zero_outs.append(np.zeros(shape, dtype))
    n_params = len(in_names)
    all_in_names = list(in_names) + list(out_names)
    if nc.partition_id_tensor is not None:
        all_in_names.append(nc.partition_id_tensor.name)

    def _body(*args):
        operands = list(args)
        if nc.partition_id_tensor is not None:
            operands.append(bass2jax.partition_id_tensor())
        return tuple(bass2jax._bass_exec_p.bind(
            *operands,
            out_avals=tuple(out_avals),
            in_names=tuple(all_in_names),
            out_names=tuple(out_names),
            lowering_input_output_aliases=(),
            sim_require_finite=True,
            sim_require_nnan=True,
            nc=nc,
        ))

    devices = jax.devices()[:NCORES]
    mesh = Mesh(np.asarray(devices), ("core",))
    n_outs = len(out_avals)
    fn = jax.jit(
        shard_map(_body, mesh=mesh,
                  in_specs=(PartitionSpec("core"),) * (n_params + n_outs),
                  out_specs=(PartitionSpec("core"),) * n_outs,
                  check_rep=False),
        keep_unused=True,
    )
    sharding = jax.sharding.NamedSharding(mesh, PartitionSpec("core"))
    dev_zero = [jax.device_put(
        np.zeros((NCORES * z.shape[0], *z.shape[1:]), z.dtype), sharding)
        for z in zero_outs]

    def run(in_maps):
        concat_in = [np.concatenate([np.asarray(m[name]) for m in in_maps], axis=0)
                     for name in in_names]
        dev_in = [jax.device_put(a, sharding) for a in concat_in]
        outs = fn(*dev_in, *dev_zero)
        named = dict(zip(out_names, outs))
        yf = np.asarray(named["y"]).reshape(NCORES, C, N)
        ybt = np.asarray(named["yb"], dtype=np.float32).reshape(NCORES, C, ICH)
        yf[:, :, N - ICH:N] = ybt
        return yf

    return run


def kernel(x, g, w_qkv, w_out, b_out):
    global _RUNNER_CACHE
    nc = _build()
    in_maps = _host_prep(x, g, w_qkv, w_out)
    try:
        if _RUNNER_CACHE is None:
            _RUNNER_CACHE = _make_runner(nc)
        y_cores = _RUNNER_CACHE(in_maps)
    except Exception:
        res = run_bass_kernel_spmd(nc, in_maps, core_ids=list(range(NCORES)))
        y_cores = np.stack([res.results[c]["y"] for c in range(NCORES)])
        ybt = np.stack([np.asarray(res.results[c]["yb"], dtype=np.float32)
                        for c in range(NCORES)])
        y_cores[:, :, N - ICH:N] = ybt
    y = np.zeros((B, C, N), np.float32)
    for core in range(NCORES):
        y[core // 4] += y_cores[core]
    y += np.asarray(b_out, np.float32)[None, :, None]
    return y.reshape(B, C, H, W, D)
